# revision 49
# baseline (speedup 1.0000x reference)
"""MultiHeadAttention (B=2, S=2048, D=1024, 16 heads, causal, torch-.view head
split) on 8 TRN2 NeuronCores.

Sharding: core c handles batch b = c//4 and heads [4g, 4g+4) with g = c%4
(head h only touches token rows [128h, 128(h+1)) of its batch, so each core
needs just 512 rows of q/k/v). Wp is row-sharded by head; each core returns a
partial (2048, 1024) output (bf16) and the host sums the 4 partials per batch
in f32.

Layout notes:
- Head h's (2048, 64) matrices come from the (128 tokens x 1024 cols) block
  via s = 16*t + c, d = col%64, c = col//64. On-chip we keep head-space
  sequence order PERMUTED within each 128-tile: w = 8*c + t_lo (t = 8*j+t_lo),
  which makes all gather copies contiguous while preserving the causal
  block structure. The final output DMA un-permutes.
- Scheduling: attention q-chunks run in DESCENDING order so the scalar
  engine's exp work (the secondary bottleneck) starts as early as possible,
  overlapped with the V projection which is interleaved into the PE stream.
- S^T matmuls for the two heads of a pair use disjoint partition halves and
  execute concurrently on the PE (row_grp h0/h64); their exps are fused into
  one 2-bank-wide activation.
"""

import numpy as np
import ml_dtypes
from contextlib import ExitStack

import concourse.bass as bass
import concourse.tile as tile
from concourse import bacc, mybir
from concourse.bass_utils import run_bass_kernel_spmd
from concourse.masks import make_identity

F32 = mybir.dt.float32
F16 = mybir.dt.float16
F16_NP = np.float16
BF16 = mybir.dt.bfloat16
BF16_NP = ml_dtypes.bfloat16

B, S, D, NH, HD = 2, 2048, 1024, 16, 64
HPC = 4          # heads per core
ROWS = 512       # token rows per core
N_CORES = 8
EXP_FN = mybir.ActivationFunctionType.Exp


def _perm_mask_np():
    """(128,128) bf16 mask in permuted within-tile coords: mask[wk, wq] = 1
    iff s(wq) >= s(wk), with s(w) = 16*(w%8) + w//8."""
    w = np.arange(128)
    s = 16 * (w % 8) + w // 8
    m = (s[None, :] >= s[:, None]).astype(np.float32)
    return m.astype(BF16_NP)


_PROGRAM = None


def _build_program(debug_dump=False, trunc=None):
    nc = bacc.Bacc("TRN2", target_bir_lowering=False, debug=False)

    # all inputs pre-laid-out host-side to match SBUF tiles exactly, so
    # every DMA is >=2KB-contiguous per partition line
    qT_d = nc.dram_tensor("qT", [128, 8, ROWS], F16, kind="ExternalInput").ap()
    kT_d = nc.dram_tensor("kT", [128, 8, ROWS], F16, kind="ExternalInput").ap()
    vT_d = nc.dram_tensor("vT", [128, 8, ROWS], BF16, kind="ExternalInput").ap()
    Wq_d = nc.dram_tensor(
        "Wq", [8, 128, 8, 128], F16, kind="ExternalInput"
    ).ap()
    Wk_d = nc.dram_tensor(
        "Wk", [8, 128, 8, 128], F16, kind="ExternalInput"
    ).ap()
    Wv_d = nc.dram_tensor(
        "Wv", [8, 128, 8, 128], BF16, kind="ExternalInput"
    ).ap()
    Wp_d = nc.dram_tensor("Wp", [128, 2, D], BF16, kind="ExternalInput").ap()
    mask_d = nc.dram_tensor("mask", [128, 128], BF16, kind="ExternalInput").ap()
    out_d = nc.dram_tensor("out", [S, D], BF16, kind="ExternalOutput").ap()
    dbg = {}
    if debug_dump:
        for nm, shape, dt in [
            ("dKT", [128, 8, ROWS], F16),
            ("dQhT", [128, 2, 16, 16, 8], F16),
            ("dKhT", [128, 2, 16, 16, 8], F16),
            ("dVnat", [128, HPC, 16, HD + 1], BF16),
            ("dPT0", [128, 2, 16, ROWS], BF16),
            ("dPT1", [128, 2, 16, ROWS], BF16),
            ("dattT2", [128, 2, 16, 128], BF16),
        ]:
            dbg[nm] = nc.dram_tensor(nm, shape, dt, kind="ExternalOutput").ap()

    with tile.TileContext(nc) as tc:
        with ExitStack() as ctx:
            # ---------------- persistent SBUF ----------------
            pers = ctx.enter_context(tc.tile_pool(name="pers", bufs=1))
            # projected X^T, block layout: [p, dblk, t] = X^T[128*dblk+p, t]
            QT_sb = pers.tile([128, 8, ROWS], F16, tag="QT_sb")
            KT_sb = pers.tile([128, 8, ROWS], F16, tag="KT_sb")
            VT_sb = pers.tile([128, 8, ROWS], BF16, tag="VT_sb")
            # partition-half-swapped copies
            QT_sw = pers.tile([128, 8, ROWS], F16, tag="QT_sw")
            KT_sw = pers.tile([128, 8, ROWS], F16, tag="KT_sw")
            VT_sw = pers.tile([128, 8, ROWS], BF16, tag="VT_sw")
            # head-gathered, pair-packed: [64*(h%2)+d, h//2, j, c, t_lo]
            QhT = pers.tile([128, 2, 16, 16, 8], F16, tag="QhT")
            KhT = pers.tile([128, 2, 16, 16, 8], F16, tag="KhT")
            V_pre = pers.tile([128, 2, 16, 16, 8], BF16, tag="V_pre")
            # V natural per head + ones column: [w, hl, j, 0:65]
            V_nat = pers.tile([128, HPC, 16, HD + 1], BF16, tag="V_nat")
            # exp'd S^T, double-buffered by qc parity: [kpos, half, kt, q]
            PT = [
                pers.tile(
                    [128, 2, 16, ROWS], BF16, tag=f"PT{i}", name=f"PT{i}"
                )
                for i in range(2)
            ]
            # att^T pair-packed for Wp: [64*(h%2)+d, h//2, qt, wq]
            attT2 = pers.tile([128, 2, 16, 128], BF16, tag="attT2")
            Wp_sb = pers.tile([128, 2, D], BF16, tag="Wp_sb")
            mask_t = pers.tile([128, 128], BF16, tag="mask_t")
            ident = pers.tile([128, 128], BF16, tag="ident")
            make_identity(nc, ident)

            # Non-weight DMAs go through the Pool engine's DGE so the SP
            # hardware-queue rotation carries only the 24 wcol DMAs: with
            # wcol bufs=8 each slot-reuse DMA lands on the same queue as its
            # predecessor, keeping sync-wait counts within the HW limit.
            nc.gpsimd.dma_start(out=mask_t, in_=mask_d)
            nc.gpsimd.dma_start(out=Wp_sb, in_=Wp_d)
            nc.gpsimd.memset(V_nat[:, :, :, HD : HD + 1], 1.0)

            # ---------------- PSUM pools ----------------
            # psE: [128,2,512] f32 2-bank tiles shared by projections, S^T
            # pairs (fused 2-bank exp) and out-proj. psPV: [128,2,128]
            # tiles, two 512B-aligned accumulator slots each. psT: one
            # bank for the transposes.
            psE = ctx.enter_context(tc.tile_pool(name="psE", bufs=2, space="PSUM"))
            psPV_pool = ctx.enter_context(
                tc.tile_pool(name="psPV", bufs=2, space="PSUM")
            )
            psT = ctx.enter_context(tc.tile_pool(name="psT", bufs=1, space="PSUM"))

            xin_pool = ctx.enter_context(tc.tile_pool(name="xin", bufs=3))
            w_pool = ctx.enter_context(tc.tile_pool(name="wcol", bufs=8))
            an_pool = ctx.enter_context(tc.tile_pool(name="attn", bufs=8))
            sm_pool = ctx.enter_context(tc.tile_pool(name="small", bufs=8))
            out_pool = ctx.enter_context(tc.tile_pool(name="outt", bufs=4))

            # ---------------- helpers ----------------
            def gather_batch(dst, src_sb, src_sw):
                """Head gather dst[64par+d, hp, j, c, tl] =
                src[64(c%2)+d, c//2, 128hl + 8j + tl], reading the
                half-swapped copy when par != c%2. All on DVE (4x mode)."""
                for hl in range(HPC):
                    par, hp = hl % 2, hl // 2
                    po = 64 * par
                    for c0 in range(2):
                        srct = src_sb if par == c0 else src_sw
                        inv = srct[
                            po : po + 64, :, 128 * hl : 128 * (hl + 1)
                        ].rearrange("d a (j w) -> d j a w", w=8)
                        outv = dst[po : po + 64, hp].rearrange(
                            "d j (cc c2) w -> d j cc c2 w", c2=2
                        )[:, :, :, c0, :]
                        nc.vector.tensor_copy(outv, inv)

            def emit_swap(xt_sw, xt_out, dblk, eng):
                a, b = dblk, dblk + 1
                eng.dma_start(out=xt_sw[0:64, a:b], in_=xt_out[64:128, a:b])
                eng.dma_start(out=xt_sw[64:128, a:b], in_=xt_out[0:64, a:b])

            def proj_pair(x_in, wd, xt_out, xt_sw, pair, xdt, copy_eng, swap_eng=nc.gpsimd):
                """Project W col-blocks 2*pair, 2*pair+1 into
                xt_out[:, 2p:2p+2, :] via one 2-bank psum tile."""
                ps = psE.tile([128, 2, ROWS], F32, tag="psE")
                for half in range(2):
                    dblk = 2 * pair + half
                    wcol = w_pool.tile([128, 8, 128], xdt, tag="wcol")
                    nc.sync.dma_start(out=wcol, in_=wd[dblk])
                    for mt in range(8):
                        nc.tensor.matmul(
                            ps[:, half, :],
                            lhsT=wcol[:, mt, :],
                            rhs=x_in[:, mt, :],
                            start=(mt == 0),
                            stop=(mt == 7),
                        )
                if copy_eng is nc.scalar:
                    copy_eng.copy(xt_out[:, 2 * pair : 2 * pair + 2, :], ps)
                else:
                    copy_eng.tensor_copy(
                        xt_out[:, 2 * pair : 2 * pair + 2, :], ps
                    )
                emit_swap(xt_sw, xt_out, 2 * pair, swap_eng)
                emit_swap(xt_sw, xt_out, 2 * pair + 1, swap_eng)

            mask_rr = [0]

            def st_pair(qc, hp, kt):
                """S^T matmuls for both heads of pair hp (concurrent row
                groups) + one fused 2-bank exp into PT[qc%2]; mask on the
                diagonal tiles (alternating DVE / GpSimd)."""
                qoff = max(0, 128 * kt - 512 * qc)
                pt = PT[hp]
                ps = psE.tile([128, 2, ROWS], F32, tag="psE")
                for half in range(2):
                    ho = 64 * half
                    nc.tensor.matmul(
                        ps[:, half, qoff:ROWS],
                        lhsT=KhT[ho : ho + 64, hp, kt, :, :],
                        rhs=QhT[
                            ho : ho + 64, hp,
                            4 * qc + qoff // 128 : 4 * (qc + 1), :, :,
                        ],
                        start=True,
                        stop=True,
                    )
                nc.scalar.activation(
                    pt[:, :, kt, qoff:ROWS], ps[:, :, qoff:ROWS], EXP_FN
                )
                if kt >= 4 * qc:  # diagonal tile
                    for half in range(2):
                        eng = nc.vector if mask_rr[0] % 2 == 0 else nc.gpsimd
                        mask_rr[0] += 1
                        eng.tensor_mul(
                            pt[:, half, kt, qoff : qoff + 128],
                            pt[:, half, kt, qoff : qoff + 128],
                            mask_t,
                        )

            pv_rr = [0]

            def pv_step(qc, hp, s):
                """P@V chains for both heads of the pair at q-subtile s,
                normalization, and transpose into attT2."""
                nkt = 4 * qc + s + 1
                pt = PT[hp]
                acc = psPV_pool.tile([128, 2, 128], F32, tag="psPV", name="acc")
                for half in range(2):
                    hl = 2 * hp + half
                    for kt in range(nkt):
                        nc.tensor.matmul(
                            acc[:, half, 0 : HD + 1],
                            lhsT=pt[:, half, kt, 128 * s : 128 * (s + 1)],
                            rhs=V_nat[:, hl, kt, :],
                            start=(kt == 0),
                            stop=(kt == nkt - 1),
                        )
                recip = sm_pool.tile([128, 2], F32, tag="recip")
                nc.vector.reciprocal(recip, acc[:, :, HD : HD + 1])
                attn2 = an_pool.tile([128, 128], BF16, tag="attn2")
                for half in range(2):
                    nc.vector.tensor_scalar_mul(
                        attn2[:, 64 * half : 64 * (half + 1)],
                        acc[:, half, 0:HD],
                        recip[:, half : half + 1],
                    )
                return attn2

            def pv_finish(qc, hp, s, attn2):
                ps_t = psT.tile([128, 128], BF16, tag="psT")
                nc.tensor.transpose(ps_t, attn2, ident)
                nc.vector.tensor_copy(attT2[:, hp, 4 * qc + s, :], ps_t)

            def outproj(qt):
                """Output projection for one 128-row q tile; bf16 partial."""
                ot = out_pool.tile([128, 2, ROWS], BF16, tag="out_t")
                po = psE.tile([128, 2, ROWS], F32, tag="psE")
                for ec in range(2):
                    for pair in range(2):
                        nc.tensor.matmul(
                            po[:, ec, :],
                            lhsT=attT2[:, pair, qt, :],
                            rhs=Wp_sb[:, pair, 512 * ec : 512 * (ec + 1)],
                            start=(pair == 0),
                            stop=(pair == 1),
                        )
                nc.vector.tensor_copy(ot, po)
                # un-permute rows: partition w=8c+tl -> row 16*tl+c.
                dst = out_d[128 * qt : 128 * (qt + 1), :].rearrange(
                    "(tl c) e -> c tl e", tl=8
                )
                nc.sync.dma_start(out=dst, in_=ot)

            def v_transpose_group(hl_lo, jg):
                """Transpose V_pre -> V_nat for heads (hl_lo, hl_lo+1) and
                j in (2jg, 2jg+1): 4 row-group-paired PE transposes."""
                for sidx in range(4):
                    j = 2 * jg + sidx // 2
                    hl = hl_lo + sidx % 2
                    hp, ho = hl // 2, (hl % 2) * 64
                    ps_v = psT.tile([128, HD], BF16, tag="psT")
                    nc.tensor.transpose(
                        ps_v,
                        V_pre[ho : ho + 64, hp, j, :, :],
                        ident[ho : ho + 64, ho : ho + 64],
                    )
                    nc.vector.tensor_copy(V_nat[:, hl, j, 0:HD], ps_v)

            def interleave(sts, fillers):
                """Emit st units evenly spread between filler units so the
                PE filler work absorbs the scalar exp pacing."""
                if not fillers:
                    for f in sts:
                        f()
                    return
                ratio = len(sts) / len(fillers)
                si = 0
                for i, f in enumerate(fillers):
                    take = round((i + 1) * ratio) - round(i * ratio)
                    for sfn in sts[si : si + take]:
                        sfn()
                    si += take
                    f()
                for sfn in sts[si:]:
                    sfn()

            kwcols = {}
            # ---------------- emission ----------------
            # All three x^T input DMAs go out first on the striped sync
            # path, so the SP queue rotation after them is pure wcol DMAs.
            x_k = xin_pool.tile([128, 8, ROWS], F16, tag="x_in")
            nc.sync.dma_start(out=x_k[:, :, 0:256], in_=kT_d[:, :, 0:256])
            nc.sync.dma_start(out=x_k[:, :, 256:512], in_=kT_d[:, :, 256:512])
            x_q = xin_pool.tile([128, 8, ROWS], F16, tag="x_in")
            nc.scalar.dma_start(
                out=x_q, in_=qT_d
            )
            x_v = xin_pool.tile([128, 8, ROWS], BF16, tag="x_in")
            nc.gpsimd.dma_start(
                out=x_v, in_=vT_d
            )

            # K projection (scalar copies), swap, gather. The first
            # chains only need the first half of x_k (tokens 0:256), so the
            # PE starts ~5us earlier.
            for pair in range(4):
                ps = psE.tile([128, 2, ROWS], F32, tag="psE", name="psK")
                for th in range(2):
                    for half in range(2):
                        dblk = 2 * pair + half
                        if th == 0:
                            wcol = w_pool.tile(
                                [128, 8, 128], F16, tag="wcol", name="wcolK"
                            )
                            nc.sync.dma_start(out=wcol, in_=Wk_d[dblk])
                            kwcols[dblk] = wcol
                        wc = kwcols[dblk]
                        for mt in range(8):
                            nc.tensor.matmul(
                                ps[:, half, 256 * th : 256 * (th + 1)],
                                lhsT=wc[:, mt, :],
                                rhs=x_k[:, mt, 256 * th : 256 * (th + 1)],
                                start=(mt == 0),
                                stop=(mt == 7),
                            )
                nc.scalar.copy(KT_sb[:, 2 * pair : 2 * pair + 2, :], ps)
                emit_swap(KT_sw, KT_sb, 2 * pair, nc.gpsimd)
                emit_swap(KT_sw, KT_sb, 2 * pair + 1, nc.gpsimd)
            gather_batch(KhT, KT_sb, KT_sw)

            # Q projection (scalar copies), swap, gather
            for pair in range(4):
                proj_pair(x_q, Wq_d, QT_sb, QT_sw, pair, F16, nc.scalar)
            gather_batch(QhT, QT_sb, QT_sw)

            # V projection (vector copies) as filler among S^T(3,0)
            proj_pair(x_v, Wv_d, VT_sb, VT_sw, 0, BF16, nc.vector)
            interleave(
                [lambda kt=kt: st_pair(3, 0, kt) for kt in range(16)],
                [
                    lambda p=p: proj_pair(
                        x_v, Wv_d, VT_sb, VT_sw, p, BF16, nc.vector
                    )
                    for p in (1, 2, 3)
                ],
            )
            gather_batch(V_pre, VT_sb, VT_sw)

            # S^T(3,1) with the V transposes as (late) filler
            vtgs = [
                lambda hl_lo=hl_lo, jg=jg: v_transpose_group(hl_lo, jg)
                for hl_lo in (0, 2)
                for jg in range(8)
            ]
            for kt in range(8):
                st_pair(3, 1, kt)
            interleave(
                [lambda kt=kt: st_pair(3, 1, kt) for kt in range(8, 16)], vtgs
            )

            if trunc == "proj":
                nc.compile()
                return nc

            # Attention pipeline, qc descending. PT is double-buffered by
            # head pair. S^T(qc-1, hp0) interleaves with PV chains of
            # (qc, hp1) (disjoint PT buffers), and S^T(qc-1, hp1) with
            # outproj(qc): the chain/outproj matmuls fill the PE while the
            # scalar engine's exp lags behind the S^T pairs.
            for qc in (3, 2, 1, 0):
                stq = qc - 1
                a0 = [pv_step(qc, 0, s) for s in range(4)]
                sts0 = (
                    [
                        lambda kt=kt: st_pair(stq, 0, kt)
                        for kt in range(4 * stq + 4)
                    ]
                    if stq >= 0
                    else []
                )
                sts1 = (
                    [
                        lambda kt=kt: st_pair(stq, 1, kt)
                        for kt in range(4 * stq + 4)
                    ]
                    if stq >= 0
                    else []
                )
                a1 = []
                fillD = [
                    lambda s=s: a1.append(pv_step(qc, 1, s)) for s in range(4)
                ] + [
                    lambda s=s: pv_finish(qc, 0, s, a0[s]) for s in range(4)
                ]
                interleave(sts0, fillD)
                fillF = [
                    lambda s=s: pv_finish(qc, 1, s, a1[s]) for s in range(4)
                ] + [lambda qt=qt: outproj(qt) for qt in range(4 * qc, 4 * qc + 4)]
                interleave(sts1, fillF)

            if debug_dump:
                nc.sync.dma_start(out=dbg["dKT"], in_=KT_sb)
                nc.sync.dma_start(out=dbg["dQhT"], in_=QhT)
                nc.sync.dma_start(out=dbg["dKhT"], in_=KhT)
                nc.sync.dma_start(out=dbg["dVnat"], in_=V_nat)
                nc.sync.dma_start(out=dbg["dPT0"], in_=PT[0])
                nc.sync.dma_start(out=dbg["dPT1"], in_=PT[1])
                nc.sync.dma_start(out=dbg["dattT2"], in_=attT2)

    nc.compile()
    return nc


def get_program(debug_dump=False, trunc=None):
    global _PROGRAM
    if _PROGRAM is None:
        _PROGRAM = _build_program(debug_dump, trunc)
    return _PROGRAM


def _w_layout(W, dt):
    # [dblk, p, a, dcol]: element (dblk, p, a, d) = W[128a+p, 128dblk+d]
    W = np.asarray(W, np.float32).reshape(8, 128, 8, 128)
    return np.ascontiguousarray(W.transpose(2, 1, 0, 3)).astype(dt)


def _x_layout(x, dt):
    # [p, a, t]: element (p, a, t) = x[t, 128a+p]
    xT = np.asarray(x, np.float32).T.reshape(8, 128, ROWS)
    return np.ascontiguousarray(xT.transpose(1, 0, 2)).astype(dt)


def make_in_maps(q, k, v, Wq, Wk, Wv, Wp):
    mask = _perm_mask_np()
    Wq_b = _w_layout(Wq, F16_NP)
    Wk_b = _w_layout(Wk, F16_NP)
    Wv_b = _w_layout(Wv, BF16_NP)
    Wp_f = np.asarray(Wp, np.float32)
    in_maps = []
    for core in range(N_CORES):
        b, g = divmod(core, 4)
        rows = slice(ROWS * g, ROWS * (g + 1))
        Wp_g = Wp_f[HPC * HD * g : HPC * HD * (g + 1)].reshape(2, 128, D)
        in_maps.append(
            {
                "qT": _x_layout(np.asarray(q[b], np.float32)[rows], F16_NP),
                "kT": _x_layout(np.asarray(k[b], np.float32)[rows], F16_NP),
                "vT": _x_layout(np.asarray(v[b], np.float32)[rows], BF16_NP),
                "Wq": Wq_b,
                "Wk": Wk_b,
                "Wv": Wv_b,
                "Wp": np.ascontiguousarray(Wp_g.transpose(1, 0, 2)).astype(
                    BF16_NP
                ),
                "mask": mask,
            }
        )
    return in_maps


def kernel(q, k, v, Wq, Wk, Wv, Wp, _trace=False, _trace_kwargs=None):
    nc = get_program()
    in_maps = make_in_maps(q, k, v, Wq, Wk, Wv, Wp)
    res = run_bass_kernel_spmd(
        nc,
        in_maps,
        core_ids=list(range(N_CORES)),
        trace=_trace,
        **(_trace_kwargs or {}),
    )
    outs = [
        np.asarray(res.results[c]["out"], np.float32) for c in range(N_CORES)
    ]
    full = np.stack(
        [
            outs[0] + outs[1] + outs[2] + outs[3],
            outs[4] + outs[5] + outs[6] + outs[7],
        ]
    ).astype(np.float32)
    if _trace:
        kernel._last_result = res
    return full
